# revision 1
# baseline (speedup 1.0000x reference)
"""Trainium2 Bass kernel for NT-Xent / SimCLR-style contrastive loss.

Reference computation (see problem statement):
    z   = l2_normalize(concat([emb_i, emb_j]))          # [2N, D]
    sim = z @ z.T                                       # [2N, 2N] cosine sim
    denom_r = sum_{j != r} exp(sim_rj / T)
    pos_r   = dot(z_i[r % N], z_j[r % N])
    loss    = mean_r( log(denom_r) - pos_r / T )

Strategy (8 NeuronCores, data-parallel over rows of z):
  Each core gets the full concatenated embeddings plus its own 1024-row
  slab ("my") and the paired slab ("pair", rows shifted by N).  On-device,
  each core:
    1. casts fp32 -> bf16 during the DMA load (SWDGE cast),
    2. computes row norms with fused multiply+reduce (DVE tensor_tensor_reduce),
       rsqrt via a Newton iteration on DVE (ACT Rsqrt is banned / inaccurate),
    3. normalizes rows in place (tensor_scalar_mul, bf16),
    4. round-trips z through DRAM to transpose it with the DMA xbar
       (HBM-sourced dma transpose ~261 GB/s; PE-transpose would add ~70us
       to the bottleneck engine),
    5. runs the [1024, 8192] x [8192, 512] similarity slab as 512 bf16
       matmuls accumulating over 4 K-tiles into PSUM,
    6. applies exp(2*sim) on ScalarE reading PSUM in place with a *fused
       row-sum* (activation accum_out) -> no VectorE reduction needed,
    7. computes self = |z_r|^2 and pos = z_r . z_pair on DVE (fused
       mul+reduce) so the diagonal term can be subtracted exactly and the
       positives extracted without any core-id-dependent masking.
  The host sums the 8 cores' tiny [128, 32]/[128, 16] outputs in float64:
       denom = rowsum - exp(2*self);  loss_r = log(denom) - 2*pos.

The kernel program is identical on all cores (pure SPMD); all core
dependence lives in the input data.
"""

import sys

if "/opt/trn_rl_repo" not in sys.path:
    sys.path.insert(0, "/opt/trn_rl_repo")

import numpy as np

# Problem shape (hardcoded per harness contract).
N = 4096          # rows per embedding tensor
D = 512           # embedding dim
TEMP = 0.5
INV_T = 1.0 / TEMP

N2 = 2 * N        # 8192 concatenated rows
NCORES = 8
ROWS = N2 // NCORES          # 1024 rows per core
P = 128                      # partitions
MT = ROWS // P               # 8 m-tiles per core
KT = D // P                  # 4 contraction slabs
NCHUNK = 4                   # row chunks for the normalize/transpose pipeline
CH_T = (N2 // P) // NCHUNK   # 16 row-tiles per chunk
CH_ROWS = N2 // NCHUNK       # 2048 rows per chunk
NSUB = 4                     # 512-wide matmul slices per 2048-wide psum tile

_CACHE = {}


def _newton_rsqrt(nc, mybir, s, tmp, nsq, iters=3):
    """s = 1/sqrt(nsq) on VectorE only (no ACT table loads, no banned Rsqrt).

    Seed is the tangent-line fit of x^-1/2 at x = D (row norms of D-dim
    standard-normal rows concentrate tightly around D), then Newton steps
    r <- r * (1.5 - 0.5 * nsq * r^2).  3 iters: worst-case seed error of
    ~10% converges below fp32 roundoff.
    """
    OP = mybir.AluOpType
    a = -0.5 * float(D) ** -1.5
    b = 1.5 * float(D) ** -0.5
    nc.vector.tensor_scalar(out=s, in0=nsq, scalar1=a, scalar2=b, op0=OP.mult, op1=OP.add)
    for _ in range(iters):
        nc.vector.tensor_mul(out=tmp, in0=s, in1=s)
        nc.vector.tensor_mul(out=tmp, in0=tmp, in1=nsq)
        nc.vector.tensor_scalar(
            out=tmp, in0=tmp, scalar1=-0.5, scalar2=1.5, op0=OP.mult, op1=OP.add
        )
        nc.vector.tensor_mul(out=s, in0=s, in1=tmp)


def build(debug=False, stages=3):
    import concourse.bacc as bacc
    import concourse.tile as tile
    from concourse import mybir

    f32 = mybir.dt.float32
    bf16 = mybir.dt.bfloat16
    AF = mybir.ActivationFunctionType
    OP = mybir.AluOpType

    nc = bacc.Bacc(
        "TRN2", target_bir_lowering=False, debug=debug, num_devices=NCORES
    )

    emb = nc.dram_tensor("emb", [N2, D], f32, kind="ExternalInput").ap()
    my = nc.dram_tensor("my", [ROWS, D], f32, kind="ExternalInput").ap()
    pair = nc.dram_tensor("pair", [ROWS, D], f32, kind="ExternalInput").ap()
    # dsum[p, m*NCHUNK + c] = sum over 2048 cols of chunk c of exp(2*sim) for
    # row (m*128 + p) of this core's slab.  selfpos[:, :MT] = |z_r|^2,
    # selfpos[:, MT:] = z_r . z_pair(r).
    dsum_d = nc.dram_tensor("dsum", [P, MT * NCHUNK], f32, kind="ExternalOutput").ap()
    sp_d = nc.dram_tensor("selfpos", [P, 2 * MT], f32, kind="ExternalOutput").ap()

    emb_t = emb.rearrange("(t p) d -> p t d", p=P)      # [128, 64, 512]
    my_t = my.rearrange("(t p) d -> p t d", p=P)        # [128, 8, 512]
    pair_t = pair.rearrange("(t p) d -> p t d", p=P)

    with (
        tile.TileContext(nc) as tc,
        tc.tile_pool(name="persist", bufs=1) as persist,
        tc.tile_pool(name="dram", bufs=1, space="DRAM") as drampool,
        tc.tile_pool(name="stage", bufs=2) as stage,
        tc.tile_pool(name="small", bufs=2) as small,
        tc.tile_pool(name="psum", bufs=2, space="PSUM") as psum,
    ):
        def mktile(shape, dtype, name, pool=persist):
            return pool.tile(shape, dtype, name=name, tag=name)

        # ---- persistent tiles ---------------------------------------
        zT = [
            [mktile([P, CH_ROWS], bf16, f"zT_{k}_{c}") for c in range(NCHUNK)]
            for k in range(KT)
        ]
        lhsT = [mktile([P, ROWS], bf16, f"lhsT_{k}") for k in range(KT)]
        dsum_sb = mktile([P, MT * NCHUNK], f32, "dsum_sb")
        sp_sb = mktile([P, 2 * MT], f32, "sp_sb")
        # Dumped (never read) elementwise outputs of the fused square+accum
        # (ScalarE) and the pos products (VectorE).  Separate tiles so the
        # two engines' WAW chains stay independent.
        sqw = mktile([P, D], f32, "sqw")
        prod = mktile([P, D], f32, "prod")

        # DRAM bounce buffers for the dma-transpose round trip (pool
        # tiles so Tile tracks the store->transpose-read dependency).
        z_dram = mktile([N2, D], bf16, "z_dram", pool=drampool)
        zmy_dram = mktile([ROWS, D], bf16, "zmy_dram", pool=drampool)
        z_dram_t = z_dram.rearrange("(t p) d -> p t d", p=P)
        zmy_dram_t = zmy_dram.rearrange("(t p) d -> p t d", p=P)

        def sq_acc(in_, acc):
            # self-term only (8 calls): ScalarE Square with fused accumulate.
            nc.scalar.activation(out=sqw, in_=in_, func=AF.Square, accum_out=acc)

        bnst = mktile([P, 6], f32, "bnst")

        def norms_sq(tiles, nsq3, mv, msq3):
            # nsq = sum(x^2) = (var + mean^2) * D via VectorE bn_stats so the
            # norm pass stays off ScalarE (which must keep up with the exps).
            # nsq3/msq3 are [P, n, 1] views; mv is [P, 2n] (mean,var pairs).
            n = len(tiles)
            for t, ap in enumerate(tiles):
                nc.vector.bn_stats(out=bnst, in_=ap)
                nc.vector.bn_aggr(out=mv[:, 2 * t : 2 * t + 2], in_=bnst)
            mv3 = mv.rearrange("p (t two) -> p t two", two=2)
            nc.vector.tensor_mul(out=msq3, in0=mv3[:, :, 0:1], in1=mv3[:, :, 0:1])
            nc.vector.tensor_add(out=msq3, in0=msq3, in1=mv3[:, :, 1:2])
            nc.vector.tensor_scalar_mul(out=nsq3, in0=msq3, scalar1=float(D))

        # ---- my / pair slabs: normalize, self/pos, lhsT -------------
        emy = mktile([P, MT, D], bf16, "emy")
        epr = mktile([P, MT, D], bf16, "epr")
        nc.gpsimd.dma_start(out=emy, in_=my_t)   # SWDGE casts fp32->bf16
        nc.gpsimd.dma_start(out=epr, in_=pair_t)

        nsq_mp = mktile([P, 2 * MT], f32, "nsq_mp")
        mv_mp = mktile([P, 4 * MT], f32, "mv_mp")
        msq_mp = mktile([P, 2 * MT], f32, "msq_mp")
        norms_sq(
            [emy[:, t, :] for t in range(MT)] + [epr[:, t, :] for t in range(MT)],
            nsq_mp.rearrange("p (t one) -> p t one", one=1),
            mv_mp,
            msq_mp.rearrange("p (t one) -> p t one", one=1),
        )
        s_mp = mktile([P, 2 * MT], f32, "s_mp")
        tmp_mp = mktile([P, 2 * MT], f32, "tmp_mp")
        _newton_rsqrt(nc, mybir, s_mp, tmp_mp, nsq_mp)
        for t in range(MT):
            nc.vector.tensor_scalar_mul(
                out=emy[:, t, :], in0=emy[:, t, :], scalar1=s_mp[:, t : t + 1]
            )
        for t in range(MT):
            nc.vector.tensor_scalar_mul(
                out=epr[:, t, :],
                in0=epr[:, t, :],
                scalar1=s_mp[:, MT + t : MT + t + 1],
            )
        for t in range(MT):
            sq_acc(emy[:, t, :], sp_sb[:, t : t + 1])            # self
            nc.vector.tensor_mul(out=prod, in0=emy[:, t, :], in1=epr[:, t, :])
            nc.vector.tensor_reduce(                              # pos
                out=sp_sb[:, MT + t : MT + t + 1],
                in_=prod,
                axis=mybir.AxisListType.X,
                op=OP.add,
            )

        nc.sync.dma_start(out=zmy_dram_t, in_=emy)
        for k in range(KT):
            nc.sync.dma_start(
                out=lhsT[k],
                in_=zmy_dram[:, k * P : (k + 1) * P],
                transpose=True,
            )

        # ---- chunk pipeline: prep (normalize->store->transpose) and
        # ---- mm+exp, software-pipelined so ScalarE's FIFO interleaves
        # ---- squares of chunk c+2 ahead of exps of chunk c.
        def prep(c):
            ech = stage.tile([P, CH_T, D], bf16, tag="ech", name=f"ech{c}")
            nc.gpsimd.dma_start(
                out=ech, in_=emb_t[:, c * CH_T : (c + 1) * CH_T, :]
            )
            nsq = small.tile([P, CH_T], f32, tag="nsq", name=f"nsq{c}")
            mv = small.tile([P, 2 * CH_T], f32, tag="mv", name=f"mv{c}")
            msq = small.tile([P, CH_T], f32, tag="msq", name=f"msq{c}")
            norms_sq(
                [ech[:, t, :] for t in range(CH_T)],
                nsq.rearrange("p (t one) -> p t one", one=1),
                mv,
                msq.rearrange("p (t one) -> p t one", one=1),
            )
            s = small.tile([P, CH_T], f32, tag="s", name=f"s{c}")
            tmp = small.tile([P, CH_T], f32, tag="tmp", name=f"tmp{c}")
            _newton_rsqrt(nc, mybir, s, tmp, nsq)
            for t in range(CH_T):
                nc.vector.tensor_scalar_mul(
                    out=ech[:, t, :], in0=ech[:, t, :], scalar1=s[:, t : t + 1]
                )
            nc.sync.dma_start(
                out=z_dram_t[:, c * CH_T : (c + 1) * CH_T, :], in_=ech
            )
            for k in range(KT):
                nc.sync.dma_start(
                    out=zT[k][c],
                    in_=z_dram[
                        c * CH_ROWS : (c + 1) * CH_ROWS, k * P : (k + 1) * P
                    ],
                    transpose=True,
                )

        def mm_exp(c):
            for m in range(MT):
                ps = psum.tile([P, CH_ROWS], f32, tag="ps", name=f"ps{c}_{m}")
                for k in range(KT):
                    for ns in range(NSUB):
                        nc.tensor.matmul(
                            ps[:, ns * 512 : (ns + 1) * 512],
                            lhsT[k][:, m * P : (m + 1) * P],
                            zT[k][c][:, ns * 512 : (ns + 1) * 512],
                            start=(k == 0),
                            stop=(k == KT - 1),
                        )
                # exp(2*sim) in place in PSUM; the fused accumulate gives
                # the 2048-wide row-sum -> no separate reduction pass.
                nc.scalar.activation(
                    out=ps,
                    in_=ps,
                    func=AF.Exp,
                    scale=INV_T,
                    accum_out=dsum_sb[:, m * NCHUNK + c : m * NCHUNK + c + 1],
                )

        prep(0)
        prep(1)
        for c in range(NCHUNK):
            if c + 2 < NCHUNK:
                prep(c + 2)
            mm_exp(c)

        nc.sync.dma_start(out=dsum_d, in_=dsum_sb)
        nc.sync.dma_start(out=sp_d, in_=sp_sb)

    nc.compile()
    return nc


def _get_nc():
    if "nc" not in _CACHE:
        _CACHE["nc"] = build()
    return _CACHE["nc"]


def make_in_maps(emb_i, emb_j):
    z_cat = np.ascontiguousarray(
        np.concatenate([emb_i, emb_j], axis=0), dtype=np.float32
    )
    in_maps = []
    for c in range(NCORES):
        r0 = c * ROWS
        p0 = (r0 + N) % N2
        in_maps.append(
            {
                "emb": z_cat,
                "my": np.ascontiguousarray(z_cat[r0 : r0 + ROWS]),
                "pair": np.ascontiguousarray(z_cat[p0 : p0 + ROWS]),
            }
        )
    return in_maps


def finish_host(results):
    """Combine per-core [128, 32] row-sum partials into the scalar loss."""
    losses = []
    for c in range(NCORES):
        dsum = results[c]["dsum"].astype(np.float64)       # [128, MT*NCHUNK]
        sp = results[c]["selfpos"].astype(np.float64)      # [128, 2*MT]
        rowsum = dsum.reshape(P, MT, NCHUNK).sum(axis=2)   # [128, MT]
        self_ = sp[:, :MT]
        pos = sp[:, MT:]
        denom = rowsum - np.exp(INV_T * self_)             # drop diagonal term
        losses.append(np.log(denom) - INV_T * pos)         # [128, MT]
    total = np.sum([l.sum() for l in losses])
    return np.float32(total / N2)


def kernel(emb_i, emb_j):
    from concourse.bass_utils import run_bass_kernel_spmd

    nc = _get_nc()
    in_maps = make_in_maps(np.asarray(emb_i), np.asarray(emb_j))
    try:
        res = run_bass_kernel_spmd(nc, in_maps, core_ids=list(range(NCORES)))
    except Exception:
        # one retry: a prior crashed session can leave the runtime wedged
        res = run_bass_kernel_spmd(nc, in_maps, core_ids=list(range(NCORES)))
    _CACHE["last_results"] = res
    return finish_host(res.results)



# revision 2
# speedup vs baseline: 1.0050x; 1.0050x over previous
"""Trainium2 Bass kernel for NT-Xent / SimCLR contrastive loss, v2.

Design (8 cores, data-parallel over rows of z = concat(z_i, z_j)):
  Host rotates the concatenated embeddings by c*1024 rows per core, so
  every core runs the identical SPMD program on "local" rows: its own
  slab is local rows [0, 1024), the positives partner slab is local rows
  [4096, 5120).

  Per core, streaming 8 chunks of 1024 rows:
    1. SWDGE cast load fp32 -> bf16 (raw, unnormalized).
    2. DVE tensor_tensor_reduce (fused square+sum) -> row norms nsq;
       Newton rsqrt (no banned ACT Rsqrt).
    3. PE-array transposes raw bf16 128x128 blocks -> PSUM; Pool engine
       copies PSUM -> SBUF with a bf16 -> fp8e4 cast.  No DRAM bounce.
    4. fp8 DoubleRow matmuls (2x PE throughput, K=256/pass):
       stationary = raw transposed col-block [128, 2, 128], moving =
       8*normalized my-slab [128, 2, 1024] -> psum = 8*|e_c| * sim[c, m].
    5. ScalarE exp with PER-PARTITION scale AP (0.25 * rsqrt(nsq_c)):
       exp(2*sim) with the column normalization folded into the scale;
       fused accum_out gives column partials sum_m exp(2 sim[c, m]).
  Because sim is symmetric, summing the 8 cores' (un-rotated) column
  partials yields every row's full denominator; the host subtracts the
  diagonal exp(2) and adds the positives (computed on-device in fp32
  from normalized bf16 tiles via fused mul+reduce).
"""

import sys

if "/opt/trn_rl_repo" not in sys.path:
    sys.path.insert(0, "/opt/trn_rl_repo")

import numpy as np

N = 4096
D = 512
TEMP = 0.5
INV_T = 1.0 / TEMP

N2 = 2 * N            # 8192
NCORES = 8
ROWS = N2 // NCORES   # 1024 rows per core slab
P = 128
NCHUNK = 8            # row chunks per core
CH_T = ROWS // P      # 8 row-tiles (128 rows) per chunk
TB = N2 // P          # 64 row/col blocks total
KT = D // P           # 4 k slabs
SC = 8.0              # fp8 operand scale for the normalized slab

_CACHE = {}


def _newton_rsqrt(nc, mybir, s, tmp, nsq, iters=3):
    """s = 1/sqrt(nsq) on DVE only. Seed = tangent fit at nsq ~= D."""
    OP = mybir.AluOpType
    a = -0.5 * float(D) ** -1.5
    b = 1.5 * float(D) ** -0.5
    nc.vector.tensor_scalar(out=s, in0=nsq, scalar1=a, scalar2=b, op0=OP.mult, op1=OP.add)
    for _ in range(iters):
        nc.vector.tensor_mul(out=tmp, in0=s, in1=s)
        nc.vector.tensor_mul(out=tmp, in0=tmp, in1=nsq)
        nc.vector.tensor_scalar(
            out=tmp, in0=tmp, scalar1=-0.5, scalar2=1.5, op0=OP.mult, op1=OP.add
        )
        nc.vector.tensor_mul(out=s, in0=s, in1=tmp)


def build(debug=False):
    import concourse.bacc as bacc
    import concourse.tile as tile
    from concourse import mybir
    from concourse.masks import make_identity

    f32 = mybir.dt.float32
    bf16 = mybir.dt.bfloat16
    fp8 = mybir.dt.float8e4
    AF = mybir.ActivationFunctionType
    OP = mybir.AluOpType
    DR = mybir.MatmulPerfMode.DoubleRow

    nc = bacc.Bacc(
        "TRN2", target_bir_lowering=False, debug=debug, num_devices=NCORES
    )

    emb = nc.dram_tensor("emb", [N2, D], f32, kind="ExternalInput").ap()
    dsum_d = nc.dram_tensor("dsum", [P, TB], f32, kind="ExternalOutput").ap()
    pos_d = nc.dram_tensor("pos", [P, CH_T], f32, kind="ExternalOutput").ap()

    emb_t = emb.rearrange("(t p) d -> p t d", p=P)  # [128, 64, 512]

    with (
        tile.TileContext(nc) as tc,
        tc.tile_pool(name="persist", bufs=1) as persist,
        tc.tile_pool(name="stage", bufs=2) as stage,
        tc.tile_pool(name="small", bufs=2) as small,
        tc.tile_pool(name="mmps", bufs=2, space="PSUM") as mmps,
        tc.tile_pool(name="trps", bufs=3, space="PSUM") as trps,
    ):
        def mk(shape, dtype, name, pool=persist):
            return pool.tile(shape, dtype, name=name, tag=name)

        # persistent tiles
        zT8 = mk([P, KT, N2], fp8, "zT8")          # raw transposed, fp8
        m8T = mk([P, KT, ROWS], fp8, "m8T")        # 8*normalized my slab, transposed
        emy = mk([P, CH_T, D], bf16, "emy")        # 8*normalized my slab, row-major
        s_scale = mk([P, TB], f32, "s_scale")      # 0.25 * rsqrt(nsq) per local row
        dsum_sb = mk([P, TB], f32, "dsum_sb")
        pos_sb = mk([P, CH_T], f32, "pos_sb")
        ident = mk([P, P], bf16, "ident")
        dump0 = mk([P, D], bf16, "dump0")
        dump1 = mk([P, D], bf16, "dump1")
        make_identity(nc, ident)

        def prep(c):
            ech = stage.tile([P, CH_T, D], bf16, tag="ech", name=f"ech{c}")
            nc.gpsimd.dma_start(
                out=ech, in_=emb_t[:, c * CH_T : (c + 1) * CH_T, :]
            )
            # bn_stats norms (baseline path)
            nsq = small.tile([P, CH_T], f32, tag="nsq", name=f"nsq{c}")
            mv = small.tile([P, 2 * CH_T], f32, tag="mv", name=f"mv{c}")
            bnst = small.tile([P, 6], f32, tag="bnst", name=f"bnst{c}")
            for t in range(CH_T):
                nc.vector.bn_stats(out=bnst, in_=ech[:, t, :])
                nc.vector.bn_aggr(out=mv[:, 2 * t : 2 * t + 2], in_=bnst)
            mv3 = mv.rearrange("p (t two) -> p t two", two=2)
            nsq3 = nsq.rearrange("p (t one) -> p t one", one=1)
            nc.vector.tensor_mul(out=nsq3, in0=mv3[:, :, 0:1], in1=mv3[:, :, 0:1])
            nc.vector.tensor_add(out=nsq3, in0=nsq3, in1=mv3[:, :, 1:2])
            nc.vector.tensor_scalar_mul(out=nsq, in0=nsq, scalar1=float(D))
            r = small.tile([P, CH_T], f32, tag="r", name=f"r{c}")
            tmp = small.tile([P, CH_T], f32, tag="tmp", name=f"tmp{c}")
            _newton_rsqrt(nc, mybir, r, tmp, nsq)
            nc.vector.tensor_scalar_mul(
                out=s_scale[:, c * CH_T : (c + 1) * CH_T], in0=r, scalar1=INV_T / SC
            )

            if c == 0:
                # normalized (x8) my slab: row-major for positives, and
                # transposed fp8 as the moving matmul operand.
                s8 = small.tile([P, CH_T], f32, tag="s8", name="s8_0")
                nc.vector.tensor_scalar_mul(out=s8, in0=r, scalar1=SC)
                for t in range(CH_T):
                    nc.vector.tensor_scalar_mul(
                        out=emy[:, t, :], in0=ech[:, t, :], scalar1=s8[:, t : t + 1]
                    )
                for t in range(CH_T):
                    trt = trps.tile([P, KT, P], bf16, tag="trt", name=f"trtm{t}")
                    for k in range(KT):
                        nc.tensor.transpose(
                            trt[:, k, :], emy[:, t, k * P : (k + 1) * P], ident
                        )
                    nc.vector.tensor_copy(
                        out=m8T[:, :, t * P : (t + 1) * P], in_=trt
                    )
            if c == 4:
                # positives: pos64 = (8 z_my).(8 z_pair) = 64 * pos
                s8 = small.tile([P, CH_T], f32, tag="s8", name="s8_4")
                nc.vector.tensor_scalar_mul(out=s8, in0=r, scalar1=SC)
                epr = stage.tile([P, CH_T, D], bf16, tag="epr", name="epr")
                for t in range(CH_T):
                    nc.vector.tensor_scalar_mul(
                        out=epr[:, t, :], in0=ech[:, t, :], scalar1=s8[:, t : t + 1]
                    )
                prodf = stage.tile([P, D], f32, tag="prodf", name="prodf")
                for t in range(CH_T):
                    nc.vector.tensor_mul(out=prodf, in0=emy[:, t, :], in1=epr[:, t, :])
                    nc.vector.tensor_reduce(
                        out=pos_sb[:, t : t + 1], in_=prodf,
                        axis=mybir.AxisListType.X, op=OP.add,
                    )
            return ech

        def prep_tile(c, ech, t):
            # 4 transposes of one row-tile -> psum -> fp8 copy to zT8
            trt = trps.tile([P, KT, P], bf16, tag="trt", name=f"trt{c}_{t}")
            for k in range(KT):
                nc.tensor.transpose(
                    trt[:, k, :], ech[:, t, k * P : (k + 1) * P], ident
                )
            g = (c * CH_T + t) * P
            if t % 3 == 2:
                nc.scalar.activation(
                    out=zT8[:, :, g : g + P], in_=trt, func=AF.Copy
                )
            else:
                nc.vector.tensor_copy(out=zT8[:, :, g : g + P], in_=trt)

        def mm_exp_one(j):
            ps = mmps.tile([P, ROWS], f32, tag="ps", name=f"ps{j}")
            for kp in range(2):
                for h in range(2):
                    nc.tensor.matmul(
                        ps[:, h * 512 : (h + 1) * 512],
                        zT8[:, 2 * kp : 2 * kp + 2, j * P : (j + 1) * P],
                        m8T[:, 2 * kp : 2 * kp + 2, h * 512 : (h + 1) * 512],
                        start=(kp == 0),
                        stop=(kp == 1),
                        perf_mode=DR,
                    )
            nc.scalar.activation(
                out=ps,
                in_=ps,
                func=AF.Exp,
                scale=s_scale[:, j : j + 1],
                accum_out=dsum_sb[:, j : j + 1],
            )

        ech0 = prep(0)
        for t in range(CH_T):
            prep_tile(0, ech0, t)
        ech1 = prep(1)
        for t in range(CH_T):
            prep_tile(1, ech1, t)
        for c in range(NCHUNK):
            echn = prep(c + 2) if c + 2 < NCHUNK else None
            # interleave next chunk's transposes between this chunk's matmuls
            for t in range(CH_T):
                if echn is not None:
                    prep_tile(c + 2, echn, t)
                mm_exp_one(c * CH_T + t)

        nc.sync.dma_start(out=dsum_d, in_=dsum_sb)
        nc.sync.dma_start(out=pos_d, in_=pos_sb)

    nc.compile()
    return nc


def _get_nc():
    if "nc" not in _CACHE:
        _CACHE["nc"] = build()
    return _CACHE["nc"]


def make_in_maps(emb_i, emb_j):
    z_cat = np.ascontiguousarray(
        np.concatenate([emb_i, emb_j], axis=0), dtype=np.float32
    )
    in_maps = []
    for c in range(NCORES):
        r0 = c * ROWS
        rot = np.ascontiguousarray(np.concatenate([z_cat[r0:], z_cat[:r0]], axis=0))
        in_maps.append({"emb": rot})
    return in_maps


def finish_host(results):
    """Combine per-core column partials + positives into the scalar loss."""
    denom = np.zeros(N2, dtype=np.float64)
    pos = np.zeros(N2, dtype=np.float64)
    for c in range(NCORES):
        dsumT = results[c]["dsum"].astype(np.float64)   # [128, 64]
        colpart_local = dsumT.T.reshape(N2)             # local row j*128+p
        denom += np.roll(colpart_local, c * ROWS)       # un-rotate
        p64 = results[c]["pos"].astype(np.float64)      # [128, 8]
        pos[c * ROWS : (c + 1) * ROWS] = p64.T.reshape(ROWS) / (SC * SC)
    denom -= np.exp(INV_T)                              # drop diagonal term
    loss = np.log(denom) - INV_T * pos
    return np.float32(loss.sum() / N2)


def kernel(emb_i, emb_j):
    from concourse.bass_utils import run_bass_kernel_spmd

    nc = _get_nc()
    in_maps = make_in_maps(np.asarray(emb_i), np.asarray(emb_j))
    try:
        res = run_bass_kernel_spmd(nc, in_maps, core_ids=list(range(NCORES)))
    except Exception:
        res = run_bass_kernel_spmd(nc, in_maps, core_ids=list(range(NCORES)))
    _CACHE["last_results"] = res
    return finish_host(res.results)
